# revision 8
# baseline (speedup 1.0000x reference)
"""MoE routed-classification kernel for Trainium2 (8 NeuronCores, SPMD).

Problem: nn_DINOMIMICClassification — E=16 experts, each a 3-layer MLP
(D=1536 -> H=768 -> H=768 -> T=2, relu after layers 1/2); every sample of
the B=512 batch goes through the expert selected by head_idx[b].

Strategy (expert-parallel + host routing):
  - Each of the 8 cores owns 2 experts and receives only the samples routed
    to them (host groups samples by expert, pads each group to CAP=64
    columns; actual per-expert counts for the fixed input seed max out at 47).
  - Activations are kept transposed on-chip (features on partitions), so
    matmuls are W^T @ xT with the weight tile as the stationary operand and
    per-feature biases land on the partition dim, where the Activation
    engine's fused bias+relu applies them for free.
  - Weights are reordered on the host so every DMA is a [128, N] tile with
    large per-partition contiguous runs (max DMA descriptor efficiency).
    The kernel is weight-bandwidth bound: ~14.2 MB of fp32 weights stream
    through each core exactly once.
  - The tiny b3 bias is added on the host during unsharding.
All matmul arithmetic is fp32 (4 cycles/row on PE) to stay inside the fp32
error envelope; the routed sample count keeps PE comfortably under the DMA
roofline.
"""

import os

import numpy as np

# Model dims (hardcoded; the grading harness calls kernel() standalone).
E, B, D, H, T = 16, 512, 1536, 768, 2
NCORES = 8
EPC = E // NCORES  # experts per core = 2
CAP = 64  # per-expert routed-sample capacity (actual max is 47)
KD = D // 128  # 12 contraction tiles for layer 1
KH = H // 128  # 6 contraction tiles for layers 2/3
C2 = EPC * CAP  # routed-x columns per core

_CACHE = {}


def _build_program():
    """Build the (single, SPMD) Bass program run on every core."""
    from contextlib import ExitStack

    import concourse.mybir as mybir
    import concourse.tile as tile
    from concourse import bacc

    f32 = mybir.dt.float32
    # Bacc (not raw Bass): its compile() legalization splits multi-sem waits
    # into EventSemaphore sequencer ops — TPB instructions have a single
    # hardware wait slot and walrus rejects >1 ("Too many sync wait commands").
    nc = bacc.Bacc("TRN2")

    xg = nc.dram_tensor("xg", [128, KD, C2], f32, kind="ExternalInput")
    w1g = nc.dram_tensor("w1g", [EPC * KH, 128, KD * 128], f32, kind="ExternalInput")
    w2g = nc.dram_tensor("w2g", [EPC * KH, 128, KH * 128], f32, kind="ExternalInput")
    w3g = nc.dram_tensor("w3g", [128, EPC, KH, T], f32, kind="ExternalInput")
    b1g = nc.dram_tensor("b1g", [128, EPC * KH], f32, kind="ExternalInput")
    b2g = nc.dram_tensor("b2g", [128, EPC * KH], f32, kind="ExternalInput")
    outg = nc.dram_tensor("outg", [EPC, T, CAP], f32, kind="ExternalOutput")

    relu = mybir.ActivationFunctionType.Relu

    with tile.TileContext(nc) as tc, ExitStack() as ctx:
        const_pool = ctx.enter_context(tc.tile_pool(name="const", bufs=1))
        w1_pool = ctx.enter_context(tc.tile_pool(name="w1", bufs=2 * KH))
        w2_pool = ctx.enter_context(tc.tile_pool(name="w2", bufs=2 * KH))
        h_pool = ctx.enter_context(tc.tile_pool(name="h", bufs=2))
        o_pool = ctx.enter_context(tc.tile_pool(name="o", bufs=2))
        ps_pool = ctx.enter_context(tc.tile_pool(name="ps", bufs=4, space="PSUM"))
        ps3_pool = ctx.enter_context(tc.tile_pool(name="ps3", bufs=2, space="PSUM"))

        # Small always-resident inputs: routed activations, biases, W3.
        xsb = const_pool.tile([128, KD, C2], f32)
        nc.sync.dma_start(out=xsb, in_=xg[:, :, :])
        b1sb = const_pool.tile([128, EPC * KH], f32)
        nc.sync.dma_start(out=b1sb, in_=b1g[:, :])
        b2sb = const_pool.tile([128, EPC * KH], f32)
        nc.sync.dma_start(out=b2sb, in_=b2g[:, :])
        w3sb = const_pool.tile([128, EPC, KH, T], f32)
        nc.sync.dma_start(out=w3sb, in_=w3g[:, :, :, :])

        outs = []
        for e in range(EPC):
            # ---- layer 1: h1[h, c] = relu(sum_d W1[d, h] * xT[d, c] + b1[h])
            h1 = h_pool.tile([128, KH, CAP], f32, tag="h1")
            for mh in range(KH):
                w1t = w1_pool.tile([128, KD * 128], f32, tag="w1")
                nc.sync.dma_start(out=w1t, in_=w1g[e * KH + mh, :, :])
                ps = ps_pool.tile([128, CAP], f32, tag="ps")
                for kd in range(KD):
                    nc.tensor.matmul(
                        ps,
                        w1t[:, kd * 128 : (kd + 1) * 128],
                        xsb[:, kd, e * CAP : (e + 1) * CAP],
                        start=(kd == 0),
                        stop=(kd == KD - 1),
                    )
                nc.scalar.activation(
                    out=h1[:, mh, :],
                    in_=ps,
                    func=relu,
                    bias=b1sb[:, e * KH + mh : e * KH + mh + 1],
                    scale=1.0,
                )

            # ---- layer 2: h2[h', c] = relu(sum_h W2[h, h'] * h1[h, c] + b2[h'])
            h2 = h_pool.tile([128, KH, CAP], f32, tag="h2")
            for mh in range(KH):
                w2t = w2_pool.tile([128, KH * 128], f32, tag="w2")
                nc.sync.dma_start(out=w2t, in_=w2g[e * KH + mh, :, :])
                ps = ps_pool.tile([128, CAP], f32, tag="ps")
                for kh in range(KH):
                    nc.tensor.matmul(
                        ps,
                        w2t[:, kh * 128 : (kh + 1) * 128],
                        h1[:, kh, :],
                        start=(kh == 0),
                        stop=(kh == KH - 1),
                    )
                nc.scalar.activation(
                    out=h2[:, mh, :],
                    in_=ps,
                    func=relu,
                    bias=b2sb[:, e * KH + mh : e * KH + mh + 1],
                    scale=1.0,
                )

            # ---- layer 3: out[t, c] = sum_h W3[h, t] * h2[h, c]  (b3 on host)
            ps3 = ps3_pool.tile([T, CAP], f32, tag="ps3")
            for kh in range(KH):
                nc.tensor.matmul(
                    ps3,
                    w3sb[:, e, kh, :],
                    h2[:, kh, :],
                    start=(kh == 0),
                    stop=(kh == KH - 1),
                )
            ot = o_pool.tile([T, CAP], f32, tag="ot")
            nc.vector.tensor_copy(out=ot, in_=ps3)
            outs.append(ot)

        # Output DMAs on the gpsimd (SWDGE) queue so they never block the
        # sync-engine weight stream.
        for e, ot in enumerate(outs):
            nc.gpsimd.dma_start(out=outg[e, :, :], in_=ot)

    nc.finalize()
    return nc


def _get_program():
    if "nc" not in _CACHE:
        _CACHE["nc"] = _build_program()
    return _CACHE["nc"]


def kernel(x, head_idx, W1, b1, W2, b2, W3, b3):
    # Make sure the axon jax platform is reachable (the Bass program executes
    # via PJRT on the 8 tunneled NeuronCores).
    if os.environ.get("JAX_PLATFORMS") not in (None, ""):
        if "axon" not in os.environ["JAX_PLATFORMS"]:
            os.environ["JAX_PLATFORMS"] = ""

    from concourse.bass_utils import run_bass_kernel_spmd

    x = np.ascontiguousarray(np.asarray(x, dtype=np.float32))
    head_idx = np.asarray(head_idx, dtype=np.int32)
    W1 = np.asarray(W1, dtype=np.float32)
    b1 = np.asarray(b1, dtype=np.float32)
    W2 = np.asarray(W2, dtype=np.float32)
    b2 = np.asarray(b2, dtype=np.float32)
    W3 = np.asarray(W3, dtype=np.float32)
    b3 = np.asarray(b3, dtype=np.float32)

    # ---- host-side routing: group sample indices by expert, pad to CAP.
    idx_per_e = [np.nonzero(head_idx == e)[0] for e in range(E)]
    counts = [len(ix) for ix in idx_per_e]
    assert max(counts) <= CAP, f"expert overflow: {counts}"

    # ---- host-side weight/activation reorders into DMA-friendly layouts.
    # w1r[ge, mh, p, kd, h] = W1[ge, kd*128+p, mh*128+h]
    w1r = W1.reshape(E, KD, 128, KH, 128).transpose(0, 3, 2, 1, 4)
    w1r = np.ascontiguousarray(w1r).reshape(E, KH, 128, KD * 128)
    w2r = W2.reshape(E, KH, 128, KH, 128).transpose(0, 3, 2, 1, 4)
    w2r = np.ascontiguousarray(w2r).reshape(E, KH, 128, KH * 128)
    # w3r[ge, p, kh, t] = W3[ge, kh*128+p, t]
    w3r = np.ascontiguousarray(W3.reshape(E, KH, 128, T).transpose(0, 2, 1, 3))
    # b1r[ge, p, mh] = b1[ge, mh*128+p]
    b1r = np.ascontiguousarray(b1.reshape(E, KH, 128).transpose(0, 2, 1))
    b2r = np.ascontiguousarray(b2.reshape(E, KH, 128).transpose(0, 2, 1))

    in_maps = []
    for c in range(NCORES):
        ge0 = c * EPC
        xgc = np.zeros((128, KD, C2), np.float32)
        for j in range(EPC):
            ix = idx_per_e[ge0 + j]
            if len(ix):
                # x[ix] : [n, D] -> xT tiles [128, KD, n]
                xt = x[ix].T.reshape(KD, 128, len(ix)).transpose(1, 0, 2)
                xgc[:, :, j * CAP : j * CAP + len(ix)] = xt
        in_maps.append(
            {
                "xg": xgc,
                "w1g": w1r[ge0 : ge0 + EPC].reshape(EPC * KH, 128, KD * 128),
                "w2g": w2r[ge0 : ge0 + EPC].reshape(EPC * KH, 128, KH * 128),
                "w3g": np.ascontiguousarray(
                    w3r[ge0 : ge0 + EPC].transpose(1, 0, 2, 3)
                ),
                "b1g": np.ascontiguousarray(
                    b1r[ge0 : ge0 + EPC].transpose(1, 0, 2).reshape(128, EPC * KH)
                ),
                "b2g": np.ascontiguousarray(
                    b2r[ge0 : ge0 + EPC].transpose(1, 0, 2).reshape(128, EPC * KH)
                ),
            }
        )

    nc = _get_program()
    res = run_bass_kernel_spmd(nc, in_maps, core_ids=list(range(NCORES)))

    # ---- unshard: scatter per-expert outputs back to batch order, add b3.
    out = np.empty((B, T), np.float32)
    for c in range(NCORES):
        og = res.results[c]["outg"]  # [EPC, T, CAP]
        for j in range(EPC):
            ge = c * EPC + j
            ix = idx_per_e[ge]
            if len(ix):
                out[ix] = og[j, :, : len(ix)].T + b3[ge]
    return out


# revision 10
# speedup vs baseline: 1.9657x; 1.9657x over previous
"""MoE routed-classification kernel for Trainium2 (8 NeuronCores, SPMD).

Problem: nn_DINOMIMICClassification — E=16 experts, each a 3-layer MLP
(D=1536 -> H=768 -> H=768 -> T=2, relu after layers 1/2); every sample of
the B=512 batch goes through the expert selected by head_idx[b].

Strategy (expert-parallel + host routing + bf16 hi/lo arithmetic):
  - Each of the 8 cores owns 2 experts and receives only the samples routed
    to them (host groups samples by expert, pads each group to CAP=64
    columns; actual per-expert counts for the fixed input seed max out at 47).
  - fp32 matmuls on TRN2 are self-loading (no LDWEIGHTS reuse) and run at
    4 cycles/row — measured ~850ns per [128x128]x[128x64] matmul, which made
    a pure-fp32 kernel PE-bound at ~122us. Instead every fp32 value is split
    into bf16 hi + lo planes and the product is computed as
    W_hi.x_hi + W_lo.x_hi + W_hi.x_lo, all in bf16 matmuls with fp32 PSUM
    accumulation. HW-measured accuracy of this 3-term scheme: ~5e-6 relative
    (vs 2e-7 fp32, 2.4e-3 plain bf16).
  - Packing: x_hi|x_lo sit side by side in one [128, 128] moving operand, so
    each W_hi tile loads once and streams both terms (N=128); the W_lo.x_hi
    matmuls (N=64) accumulate into the same PSUM columns as the hi terms.
    A DVE add folds the x_lo half in, then the Activation engine applies
    bias+relu in one fused op (features live on partitions, so the
    per-feature bias is a legal per-partition activation bias).
  - Weight DMAs carry hi+lo planes of one [K-tile, 128-col] block per
    transfer (~786KB each, large contiguous per-partition runs). Total
    weight traffic is unchanged vs fp32: ~14.2 MB/core, the HBM roofline.
  - The tiny b3 bias is added on the host during unsharding.
"""

import os

import numpy as np

# Model dims (hardcoded; the grading harness calls kernel() standalone).
E, B, D, H, T = 16, 512, 1536, 768, 2
NCORES = 8
EPC = E // NCORES  # experts per core = 2
CAP = 64  # per-expert routed-sample capacity (actual max is 47)
KD = D // 128  # 12 contraction tiles for layer 1
KH = H // 128  # 6 contraction tiles for layers 2/3

_CACHE = {}


def _build_program():
    """Build the (single, SPMD) Bass program run on every core."""
    from contextlib import ExitStack

    import concourse.mybir as mybir
    import concourse.tile as tile
    from concourse import bacc

    f32 = mybir.dt.float32
    bf16 = mybir.dt.bfloat16
    # Bacc (not raw Bass): its compile() legalization splits multi-sem waits
    # into EventSemaphore sequencer ops — TPB instructions have a single
    # hardware wait slot and walrus rejects >1 ("Too many sync wait commands").
    nc = bacc.Bacc("TRN2")

    # xg[p, kd, e, plane, c]: plane 0 = bf16 hi, plane 1 = bf16 lo
    xg = nc.dram_tensor("xg", [128, KD, EPC, 2, CAP], bf16, kind="ExternalInput")
    # w1g[e*KH+mh, plane, p, kd*128+h] = plane of W1[ge, kd*128+p, mh*128+h]
    w1g = nc.dram_tensor("w1g", [EPC * KH, 2, 128, KD * 128], bf16, kind="ExternalInput")
    w2g = nc.dram_tensor("w2g", [EPC * KH, 2, 128, KH * 128], bf16, kind="ExternalInput")
    # w3g[p, e, plane, kh, t]
    w3g = nc.dram_tensor("w3g", [128, EPC, 2, KH, T], bf16, kind="ExternalInput")
    b1g = nc.dram_tensor("b1g", [128, EPC * KH], f32, kind="ExternalInput")
    b2g = nc.dram_tensor("b2g", [128, EPC * KH], f32, kind="ExternalInput")
    outg = nc.dram_tensor("outg", [EPC, T, CAP], f32, kind="ExternalOutput")

    relu = mybir.ActivationFunctionType.Relu

    with tile.TileContext(nc) as tc, ExitStack() as ctx:
        const_pool = ctx.enter_context(tc.tile_pool(name="const", bufs=1))
        w1_pool = ctx.enter_context(tc.tile_pool(name="w1", bufs=2 * KH))
        w2_pool = ctx.enter_context(tc.tile_pool(name="w2", bufs=2 * KH))
        h_pool = ctx.enter_context(tc.tile_pool(name="h", bufs=2))
        t_pool = ctx.enter_context(tc.tile_pool(name="t", bufs=3))
        o_pool = ctx.enter_context(tc.tile_pool(name="o", bufs=2))
        ps_pool = ctx.enter_context(tc.tile_pool(name="ps", bufs=4, space="PSUM"))
        ps3_pool = ctx.enter_context(tc.tile_pool(name="ps3", bufs=2, space="PSUM"))

        # Small always-resident inputs: routed activations, biases, W3.
        xsb = const_pool.tile([128, KD, EPC, 2, CAP], bf16)
        nc.sync.dma_start(out=xsb, in_=xg[:, :, :, :, :])
        b1sb = const_pool.tile([128, EPC * KH], f32)
        nc.sync.dma_start(out=b1sb, in_=b1g[:, :])
        b2sb = const_pool.tile([128, EPC * KH], f32)
        nc.sync.dma_start(out=b2sb, in_=b2g[:, :])
        w3sb = const_pool.tile([128, EPC, 2, KH, T], bf16)
        nc.sync.dma_start(out=w3sb, in_=w3g[:, :, :, :, :])

        outs = []
        for e in range(EPC):
            # ---- layer 1: h1[h, c] = relu(sum_d W1[d, h] * xT[d, c] + b1[h])
            h1 = h_pool.tile([128, KH, 2, CAP], bf16, tag="h1")
            for mh in range(KH):
                w1t = w1_pool.tile([128, 2, KD * 128], bf16, tag="w1")
                nc.sync.dma_start(
                    out=w1t, in_=w1g[e * KH + mh].rearrange("v p f -> p v f")
                )
                ps = ps_pool.tile([128, 2 * CAP], f32, tag="ps")
                # W_hi streams [x_hi | x_lo] (N=128); W_lo accumulates its
                # x_hi term straight into the hi columns (N=64).
                for kd in range(KD):
                    nc.tensor.matmul(
                        ps,
                        w1t[:, 0, kd * 128 : (kd + 1) * 128],
                        xsb[:, kd, e, :, :],
                        start=(kd == 0),
                        stop=False,
                    )
                for kd in range(KD):
                    nc.tensor.matmul(
                        ps[:, 0:CAP],
                        w1t[:, 1, kd * 128 : (kd + 1) * 128],
                        xsb[:, kd, e, 0, :],
                        start=False,
                        stop=(kd == KD - 1),
                    )
                # PSUM lo half -> SBUF (TensorTensor may read only one
                # PSUM operand), then bias + both halves in one DVE op,
                # relu on ACT, and a bf16 hi/lo re-split for the next layer.
                t1 = t_pool.tile([128, CAP], f32, tag="t1")
                nc.vector.tensor_copy(out=t1, in_=ps[:, CAP : 2 * CAP])
                tsum = t_pool.tile([128, CAP], f32, tag="tsum")
                nc.vector.scalar_tensor_tensor(
                    tsum,
                    ps[:, 0:CAP],
                    b1sb[:, e * KH + mh : e * KH + mh + 1],
                    t1,
                    mybir.AluOpType.add,
                    mybir.AluOpType.add,
                )
                hf = t_pool.tile([128, CAP], f32, tag="hf")
                nc.scalar.activation(out=hf, in_=tsum, func=relu)
                nc.scalar.activation(
                    out=h1[:, mh, 0, :],
                    in_=hf,
                    func=mybir.ActivationFunctionType.Copy,
                )
                nc.vector.tensor_sub(h1[:, mh, 1, :], hf, h1[:, mh, 0, :])

            # ---- layer 2: h2[h', c] = relu(sum_h W2[h, h'] * h1[h, c] + b2[h'])
            h2 = h_pool.tile([128, KH, 2, CAP], bf16, tag="h2")
            for mh in range(KH):
                w2t = w2_pool.tile([128, 2, KH * 128], bf16, tag="w2")
                nc.sync.dma_start(
                    out=w2t, in_=w2g[e * KH + mh].rearrange("v p f -> p v f")
                )
                ps = ps_pool.tile([128, 2 * CAP], f32, tag="ps")
                for kh in range(KH):
                    nc.tensor.matmul(
                        ps,
                        w2t[:, 0, kh * 128 : (kh + 1) * 128],
                        h1[:, kh, :, :],
                        start=(kh == 0),
                        stop=False,
                    )
                for kh in range(KH):
                    nc.tensor.matmul(
                        ps[:, 0:CAP],
                        w2t[:, 1, kh * 128 : (kh + 1) * 128],
                        h1[:, kh, 0, :],
                        start=False,
                        stop=(kh == KH - 1),
                    )
                t1 = t_pool.tile([128, CAP], f32, tag="t1")
                nc.vector.tensor_copy(out=t1, in_=ps[:, CAP : 2 * CAP])
                tsum = t_pool.tile([128, CAP], f32, tag="tsum")
                nc.vector.scalar_tensor_tensor(
                    tsum,
                    ps[:, 0:CAP],
                    b2sb[:, e * KH + mh : e * KH + mh + 1],
                    t1,
                    mybir.AluOpType.add,
                    mybir.AluOpType.add,
                )
                hf = t_pool.tile([128, CAP], f32, tag="hf")
                nc.scalar.activation(out=hf, in_=tsum, func=relu)
                nc.scalar.activation(
                    out=h2[:, mh, 0, :],
                    in_=hf,
                    func=mybir.ActivationFunctionType.Copy,
                )
                nc.vector.tensor_sub(h2[:, mh, 1, :], hf, h2[:, mh, 0, :])

            # ---- layer 3: out[t, c] = sum_h W3[h, t] * h2[h, c]  (b3 on host)
            ps3 = ps3_pool.tile([T, 2 * CAP], f32, tag="ps3")
            for kh in range(KH):
                nc.tensor.matmul(
                    ps3,
                    w3sb[:, e, 0, kh, :],
                    h2[:, kh, :, :],
                    start=(kh == 0),
                    stop=False,
                )
            for kh in range(KH):
                nc.tensor.matmul(
                    ps3[:, 0:CAP],
                    w3sb[:, e, 1, kh, :],
                    h2[:, kh, 0, :],
                    start=False,
                    stop=(kh == KH - 1),
                )
            t3 = o_pool.tile([T, CAP], f32, tag="t3")
            nc.vector.tensor_copy(out=t3, in_=ps3[:, CAP : 2 * CAP])
            ot = o_pool.tile([T, CAP], f32, tag="ot")
            nc.vector.tensor_add(ot, ps3[:, 0:CAP], t3)
            outs.append(ot)

        # Output DMAs on the gpsimd (SWDGE) queue so they never block the
        # sync-engine weight stream.
        for e, ot in enumerate(outs):
            nc.gpsimd.dma_start(out=outg[e, :, :], in_=ot)

    nc.finalize()
    return nc


def _get_program():
    if "nc" not in _CACHE:
        _CACHE["nc"] = _build_program()
    return _CACHE["nc"]


def _split_hilo(a):
    """fp32 array -> (hi, lo) bf16 planes with a ~= hi + lo (to ~2^-17 rel)."""
    import ml_dtypes

    hi = a.astype(ml_dtypes.bfloat16)
    lo = (a - hi.astype(np.float32)).astype(ml_dtypes.bfloat16)
    return hi, lo


def kernel(x, head_idx, W1, b1, W2, b2, W3, b3):
    # Make sure the axon jax platform is reachable (the Bass program executes
    # via PJRT on the 8 tunneled NeuronCores).
    if os.environ.get("JAX_PLATFORMS") not in (None, ""):
        if "axon" not in os.environ["JAX_PLATFORMS"]:
            os.environ["JAX_PLATFORMS"] = ""

    import ml_dtypes

    from concourse.bass_utils import run_bass_kernel_spmd

    x = np.ascontiguousarray(np.asarray(x, dtype=np.float32))
    head_idx = np.asarray(head_idx, dtype=np.int32)
    W1 = np.asarray(W1, dtype=np.float32)
    b1 = np.asarray(b1, dtype=np.float32)
    W2 = np.asarray(W2, dtype=np.float32)
    b2 = np.asarray(b2, dtype=np.float32)
    W3 = np.asarray(W3, dtype=np.float32)
    b3 = np.asarray(b3, dtype=np.float32)

    # ---- host-side routing: group sample indices by expert, pad to CAP.
    idx_per_e = [np.nonzero(head_idx == e)[0] for e in range(E)]
    counts = [len(ix) for ix in idx_per_e]
    assert max(counts) <= CAP, f"expert overflow: {counts}"

    # ---- host-side reorders into DMA-friendly layouts + bf16 hi/lo split.
    # w1r[ge, mh, p, kd, h] = W1[ge, kd*128+p, mh*128+h]
    w1r = W1.reshape(E, KD, 128, KH, 128).transpose(0, 3, 2, 1, 4)
    w1r = np.ascontiguousarray(w1r).reshape(E, KH, 128, KD * 128)
    w1hi, w1lo = _split_hilo(w1r)
    w2r = W2.reshape(E, KH, 128, KH, 128).transpose(0, 3, 2, 1, 4)
    w2r = np.ascontiguousarray(w2r).reshape(E, KH, 128, KH * 128)
    w2hi, w2lo = _split_hilo(w2r)
    # w3r[ge, p, kh, t] = W3[ge, kh*128+p, t]
    w3r = np.ascontiguousarray(W3.reshape(E, KH, 128, T).transpose(0, 2, 1, 3))
    w3hi, w3lo = _split_hilo(w3r)
    # b1r[ge, p, mh] = b1[ge, mh*128+p]
    b1r = np.ascontiguousarray(b1.reshape(E, KH, 128).transpose(0, 2, 1))
    b2r = np.ascontiguousarray(b2.reshape(E, KH, 128).transpose(0, 2, 1))

    in_maps = []
    for c in range(NCORES):
        ge0 = c * EPC
        xgc = np.zeros((128, KD, EPC, 2, CAP), ml_dtypes.bfloat16)
        for j in range(EPC):
            ix = idx_per_e[ge0 + j]
            if len(ix):
                # x[ix] : [n, D] -> xT tiles [128, KD, n]
                xt = x[ix].T.reshape(KD, 128, len(ix)).transpose(1, 0, 2)
                xhi, xlo = _split_hilo(xt)
                xgc[:, :, j, 0, : len(ix)] = xhi
                xgc[:, :, j, 1, : len(ix)] = xlo
        # [EPC, KH, 2, 128, F] with plane axis inserted
        w1c = np.stack([w1hi[ge0 : ge0 + EPC], w1lo[ge0 : ge0 + EPC]], axis=2)
        w2c = np.stack([w2hi[ge0 : ge0 + EPC], w2lo[ge0 : ge0 + EPC]], axis=2)
        # [EPC, 2, 128, KH, T] -> [128, EPC, 2, KH, T]
        w3c = np.stack([w3hi[ge0 : ge0 + EPC], w3lo[ge0 : ge0 + EPC]], axis=1)
        in_maps.append(
            {
                "xg": xgc,
                "w1g": np.ascontiguousarray(w1c).reshape(EPC * KH, 2, 128, KD * 128),
                "w2g": np.ascontiguousarray(w2c).reshape(EPC * KH, 2, 128, KH * 128),
                "w3g": np.ascontiguousarray(w3c.transpose(2, 0, 1, 3, 4)),
                "b1g": np.ascontiguousarray(
                    b1r[ge0 : ge0 + EPC].transpose(1, 0, 2).reshape(128, EPC * KH)
                ),
                "b2g": np.ascontiguousarray(
                    b2r[ge0 : ge0 + EPC].transpose(1, 0, 2).reshape(128, EPC * KH)
                ),
            }
        )

    nc = _get_program()
    res = run_bass_kernel_spmd(nc, in_maps, core_ids=list(range(NCORES)))

    # ---- unshard: scatter per-expert outputs back to batch order, add b3.
    out = np.empty((B, T), np.float32)
    for c in range(NCORES):
        og = res.results[c]["outg"]  # [EPC, T, CAP]
        for j in range(EPC):
            ge = c * EPC + j
            ix = idx_per_e[ge]
            if len(ix):
                out[ix] = og[j, :, : len(ix)].T + b3[ge]
    return out
